# revision 10
# baseline (speedup 1.0000x reference)
"""Trainium2 Bass kernel for the Digit CapsLayer (dynamic routing) problem.

Math (reference):
    u[b,c,n,d] = sum_e W[c,n,d,e] x[b,n,e]
    b0 = 0; for 3 iters: c = softmax(b, axis=c); s = sum_n c*u; v = squash(s);
    b += sum_d v*u
Output: v [B, C, D]

Strategy (pure batch-parallel over 8 cores, B=2048 -> 256/core):
  - Never materialize u. s = s0 + s_delta with
        s0[b,c,d]      = (1/3) sum_{n,e} W[c,n,d,e] x[b,n,e]   (iteration-invariant)
        s_delta[b,c,d] = sum_{n,e} W[c,n,d,e] delta[b,c,n] x[b,n,e],
        delta = softmax(b)-1/3  (tiny: |delta| ~ 1e-3, so bf16 suffices)
    b-update via z[b,c,n,e] = sum_d W[c,n,d,e] v[b,c,d] (bf16 matmul), then
        b += sum_e x*z   (elementwise + reduce over e)
  - On-chip layouts: transposed e-separated planes [n(part), b(free)];
    s0 in float32r (tf32-like) from fp32 x planes; everything
    iteration-dependent in bf16 (error contribution ~2e-5 relative).
"""

import numpy as np

import concourse.bacc as bacc
import concourse.bass as bass
import concourse.tile as tile
from concourse import mybir
from concourse.bass_utils import run_bass_kernel_spmd
from concourse.masks import make_identity

F32 = mybir.dt.float32
F32R = mybir.dt.float32r
BF16 = mybir.dt.bfloat16
NP_BF16 = mybir.dt.np(BF16)
AF = mybir.ActivationFunctionType
OP = mybir.AluOpType

B, C, N, D, E = 2048, 3, 1568, 16, 8
NCORES = 8
BC = B // NCORES          # 256 batch rows per core
HB = BC // 128            # 2 half-tiles of 128
NT = (N + 127) // 128     # 13 n-tiles (padded N = 1664)
NPAD = NT * 128
T_ROUTING = 3
CD = C * D                # 48
CDP = 96                  # padded: class c occupies partitions [32c, 32c+16)
CS = 32


def _build_module():
    nc = bacc.Bacc("TRN2", target_bir_lowering=False, debug=False)

    x_d = nc.dram_tensor("x", [HB, 128, N * E], F32, kind="ExternalInput").ap()
    ws_d = nc.dram_tensor("ws", [128, C * E * NT * D], F32, kind="ExternalInput").ap()
    ws16_d = nc.dram_tensor("ws16", [128, C * E * NT * D], BF16,
                            kind="ExternalInput").ap()
    wz16_d = nc.dram_tensor("wz16", [C, NT, D, E, 128], BF16,
                            kind="ExternalInput").ap()
    selA_d = nc.dram_tensor("selA", [CDP, C], F32, kind="ExternalInput").ap()
    selB_d = nc.dram_tensor("selB", [C, CDP], F32, kind="ExternalInput").ap()
    vout_d = nc.dram_tensor("vout", [HB, 128, CD], F32, kind="ExternalOutput").ap()

    with tile.TileContext(nc) as tc:
        from contextlib import ExitStack
        with ExitStack() as ctx:
            consts = ctx.enter_context(tc.tile_pool(name="consts", bufs=1))
            state = ctx.enter_context(tc.tile_pool(name="state", bufs=1))
            smalls = ctx.enter_context(tc.tile_pool(name="smalls", bufs=2))
            sq_psum = ctx.enter_context(
                tc.tile_pool(name="sq_psum", bufs=1, space="PSUM"))

            identity = consts.tile([128, 128], F32)
            make_identity(nc, identity)
            selA_sb = consts.tile([CDP, C], F32)
            nc.sync.dma_start(out=selA_sb, in_=selA_d)
            selB_sb = consts.tile([C, CDP], F32)
            nc.sync.dma_start(out=selB_sb, in_=selB_d)
            ws16_sb = consts.tile([128, C * E * NT * D], BF16)
            nc.sync.dma_start(out=ws16_sb, in_=ws16_d)

            # resident bf16 transposed planes: [n-part, e, t, b]
            x16 = state.tile([128, E, NT, BC], BF16)
            nc.vector.memset(x16[:, :, NT - 1, :], 0.0)

            s0_sb = state.tile([CDP, BC], F32)
            nc.vector.memset(s0_sb, 0.0)

            # ---------------- phase 1+2: load, transpose, s0 ----------------
            with ExitStack() as p12:
                ph = p12.enter_context(tc.tile_pool(name="ph12", bufs=2))
                ws_pool = p12.enter_context(tc.tile_pool(name="wsp", bufs=1))
                tp_psum = p12.enter_context(
                    tc.tile_pool(name="tp_psum", bufs=2, space="PSUM"))
                s0_psum = p12.enter_context(
                    tc.tile_pool(name="s0_psum", bufs=1, space="PSUM"))

                ws_raw = ws_pool.tile([128, C * E * NT * D], F32)
                nc.sync.dma_start(out=ws_raw, in_=ws_d)
                ws_sb = ws_pool.tile([128, C * E * NT * D], F32R)
                nc.vector.tensor_copy(out=ws_sb, in_=ws_raw)

                s0p = [s0_psum.tile([D, BC], F32, tag=f"s0_{c}", name=f"s0p_{c}")
                       for c in range(C)]

                for g in range(NT):
                    ncols = 128 if g < NT - 1 else N - 128 * (NT - 1)  # 128 or 32
                    xTg = ph.tile([128, E, BC], F32R, tag="xTg")
                    for h in range(HB):
                        xin = ph.tile([128, 128, E], F32, tag="xin")
                        nc.sync.dma_start(
                            out=xin[:, 0:ncols, :],
                            in_=x_d[h, :, g * 1024: g * 1024 + ncols * E],
                        )
                        for e in range(E):
                            tp = tp_psum.tile([128, 128], F32, tag="tp")
                            nc.tensor.transpose(
                                tp[0:ncols, :], xin[:, 0:ncols, e], identity)
                            nc.vector.tensor_copy(
                                out=xTg[0:ncols, e, h * 128:(h + 1) * 128],
                                in_=tp[0:ncols, :])
                            nc.scalar.copy(
                                out=x16[0:ncols, e, g, h * 128:(h + 1) * 128],
                                in_=tp[0:ncols, :])
                    # s0 accumulation for this n-chunk (f32r matmuls)
                    for c in range(C):
                        for e in range(E):
                            w_ap = ws_sb[0:ncols,
                                         ((c * E + e) * NT + g) * D:
                                         ((c * E + e) * NT + g + 1) * D]
                            nc.tensor.matmul(
                                s0p[c],
                                w_ap,
                                xTg[0:ncols, e, :],
                                start=(g == 0 and e == 0),
                                stop=(g == NT - 1 and e == E - 1),
                            )
                # scale by 1/3 (uniform initial coupling) while copying to SBUF
                for c in range(C):
                    nc.vector.tensor_scalar_mul(
                        out=s0_sb[c * CS:c * CS + D, :], in0=s0p[c],
                        scalar1=1.0 / 3.0)

            # ---------------- iteration pools ----------------
            soft = ctx.enter_context(tc.tile_pool(name="soft", bufs=2))
            zpool = ctx.enter_context(tc.tile_pool(name="zp", bufs=2, space="PSUM"))
            sd_psum = ctx.enter_context(
                tc.tile_pool(name="sd_psum", bufs=2, space="PSUM"))
            work = ctx.enter_context(tc.tile_pool(name="work", bufs=2))
            bstate = ctx.enter_context(tc.tile_pool(name="bstate", bufs=1))

            b_pl = [bstate.tile([128, NT, BC], F32, tag=f"b_{c}", name=f"b_pl_{c}")
                    for c in range(C)]
            s_sb = state.tile([CDP, BC], F32)
            nc.vector.memset(s_sb, 0.0)

            def squash(s_in, v16_out, v32_out):
                s2 = smalls.tile([CDP, BC], F32, tag="s2")
                nc.vector.tensor_mul(s2, s_in, s_in)
                sqp = sq_psum.tile([C, BC], F32, tag="sqp")
                nc.tensor.matmul(sqp, selA_sb, s2, start=True, stop=True)
                r = smalls.tile([C, BC], F32, tag="r")
                nc.scalar.activation(r, sqp, AF.Sqrt)
                t1 = smalls.tile([C, BC], F32, tag="t1")
                # t1 = (sq + 1) * sqrt(sq)
                nc.vector.scalar_tensor_tensor(
                    out=t1, in0=sqp, scalar=1.0, in1=r, op0=OP.add, op1=OP.mult)
                nc.vector.reciprocal(t1, t1)
                sc = smalls.tile([C, BC], F32, tag="sc")
                nc.vector.tensor_mul(sc, sqp, t1)  # sq/((1+sq)sqrt(sq))
                repp = sq_psum.tile([CDP, BC], F32, tag="repp")
                nc.tensor.matmul(repp, selB_sb, sc, start=True, stop=True)
                if v16_out is not None:
                    nc.vector.tensor_mul(v16_out, s_in, repp)
                if v32_out is not None:
                    nc.vector.tensor_mul(v32_out, s_in, repp)

            def z_pass(v16, first):
                """b += sum_e x16 * (W_z @ v) for every (c, n-chunk)."""
                for c in range(C):
                    v16c = work.tile([D, BC], BF16, tag="v16c")
                    nc.vector.tensor_copy(out=v16c, in_=v16[c * CS:c * CS + D, :])
                    for t in range(NT):
                        wz_sb = work.tile([D, E, 128], BF16, tag="wz")
                        nc.sync.dma_start(out=wz_sb, in_=wz16_d[c, t])
                        z_sb = work.tile([128, E, BC], BF16, tag="z_sb")
                        for eh in range(E // 2):
                            zp = zpool.tile([128, 2, BC], F32, tag="zp")
                            for j in range(2):
                                e = 2 * eh + j
                                nc.tensor.matmul(
                                    zp[:, j, :], wz_sb[:, e, :], v16c,
                                    start=True, stop=True)
                            nc.scalar.copy(
                                out=z_sb[:, 2 * eh:2 * eh + 2, :], in_=zp)
                        p16 = work.tile([128, E, BC], BF16, tag="p16")
                        nc.vector.tensor_mul(p16, z_sb, x16[:, :, t, :])
                        pv = p16.transpose([0, 2, 1])  # [128, b, e]
                        if first:
                            nc.vector.tensor_reduce(
                                out=b_pl[c][:, t, :], in_=pv,
                                axis=mybir.AxisListType.X, op=OP.add)
                        else:
                            accv = work.tile([128, BC], F32, tag="accv")
                            nc.vector.tensor_reduce(
                                out=accv, in_=pv,
                                axis=mybir.AxisListType.X, op=OP.add)
                            nc.vector.tensor_add(
                                b_pl[c][:, t, :], b_pl[c][:, t, :], accv)

            v16 = state.tile([CDP, BC], BF16)
            v32 = state.tile([CDP, BC], F32)

            # ---------------- iteration 1 (uniform coupling) ----------------
            squash(s0_sb, v16, None)
            z_pass(v16, first=True)

            # ---------------- iterations 2..T ----------------
            for it in range(2, T_ROUTING + 1):
                last = it == T_ROUTING
                # softmax over c; delta = softmax(b) - 1/3  (bf16)
                Zpl = soft.tile([128, NT, BC], F32, tag="Zpl", bufs=1)
                Ea = soft.tile([128, NT, BC], F32, tag="E")
                nc.scalar.activation(Ea, b_pl[0], AF.Exp)
                Eb = soft.tile([128, NT, BC], F32, tag="E")
                nc.scalar.activation(Eb, b_pl[1], AF.Exp)
                nc.vector.tensor_add(Zpl, Ea, Eb)
                Ec = soft.tile([128, NT, BC], F32, tag="E")
                nc.scalar.activation(Ec, b_pl[2], AF.Exp)
                nc.vector.tensor_add(Zpl, Zpl, Ec)
                nc.vector.reciprocal(Zpl, Zpl)

                for c in range(C):
                    Et = soft.tile([128, NT, BC], F32, tag="E")
                    nc.scalar.activation(Et, b_pl[c], AF.Exp)
                    nc.vector.tensor_mul(Et, Et, Zpl)
                    d16 = soft.tile([128, NT, BC], BF16, tag="d16")
                    nc.vector.tensor_scalar_sub(
                        out=d16, in0=Et, scalar1=1.0 / 3.0)
                    sdp = sd_psum.tile([D, BC], F32, tag="sd")
                    for e in range(E):
                        y16 = work.tile([128, NT, BC], BF16, tag="y16")
                        nc.vector.tensor_mul(y16, d16, x16[:, e, :, :])
                        for t in range(NT):
                            w_ap = ws16_sb[:, ((c * E + e) * NT + t) * D:
                                           ((c * E + e) * NT + t + 1) * D]
                            nc.tensor.matmul(
                                sdp, w_ap, y16[:, t, :],
                                start=(e == 0 and t == 0),
                                stop=(e == E - 1 and t == NT - 1))
                    nc.vector.tensor_add(
                        s_sb[c * CS:c * CS + D, :], sdp,
                        s0_sb[c * CS:c * CS + D, :])

                squash(s_sb, None if last else v16, v32 if last else None)
                if not last:
                    z_pass(v16, first=False)

            # ---------------- output ----------------
            for h in range(HB):
                vt = sq_psum.tile([128, CDP], F32, tag="vt")
                nc.tensor.transpose(
                    vt, v32[:, h * 128:(h + 1) * 128], identity[0:CDP, 0:CDP])
                vo = smalls.tile([128, C, CS], F32, tag="vo")
                nc.vector.tensor_copy(out=vo, in_=vt)
                nc.sync.dma_start(out=vout_d[h], in_=vo[:, :, 0:D])

    nc.finalize()
    return nc


def _prep_weights(W):
    """W: [1, C, N, D, E] f32 -> (ws, ws16, wz16, selA, selB)."""
    Wp = np.zeros((C, NPAD, D, E), dtype=np.float32)
    Wp[:, :N] = W[0]
    Wr = Wp.reshape(C, NT, 128, D, E)
    # ws: [128(k), C, E, NT, D]
    ws = np.ascontiguousarray(Wr.transpose(2, 0, 4, 1, 3)).reshape(128, -1)
    ws16 = ws.astype(NP_BF16)
    # wz16: [C, NT, D, E, 128(k)]
    wz16 = np.ascontiguousarray(Wr.transpose(0, 1, 3, 4, 2)).astype(NP_BF16)
    selA = np.zeros((CDP, C), dtype=np.float32)
    selB = np.zeros((C, CDP), dtype=np.float32)
    for c in range(C):
        selA[c * CS:c * CS + D, c] = 1.0
        selB[c, c * CS:c * CS + D] = 1.0
    return ws, ws16, wz16, selA, selB


_NC_CACHE = {}


def kernel(x, W):
    x = np.asarray(x, dtype=np.float32)
    W = np.asarray(W, dtype=np.float32)
    ws, ws16, wz16, selA, selB = _prep_weights(W)

    if "nc" not in _NC_CACHE:
        _NC_CACHE["nc"] = _build_module()
    nc = _NC_CACHE["nc"]

    in_maps = []
    for i in range(NCORES):
        xs = np.ascontiguousarray(
            x[i * BC:(i + 1) * BC].reshape(HB, 128, N * E))
        in_maps.append({
            "x": xs, "ws": ws, "ws16": ws16, "wz16": wz16,
            "selA": selA, "selB": selB,
        })

    res = run_bass_kernel_spmd(nc, in_maps, core_ids=list(range(NCORES)))
    out = np.empty((B, C, D), dtype=np.float32)
    for i in range(NCORES):
        out[i * BC:(i + 1) * BC] = res.results[i]["vout"].reshape(BC, C, D)
    return out
